# revision 36
# baseline (speedup 1.0000x reference)
"""Causal multi-head attention on 8 Trainium2 NeuronCores (Bass/Tile).

Problem (hardcoded): x[2,2048,1024], W_qkv[1024,3072], b_qkv[3072],
W_proj[1024,1024], b_proj[1024]; 16 heads, head_dim 64, causal softmax.

Sharding: tensor-parallel over heads — core c owns heads (2c, 2c+1).
Each core computes qkv for its 2 heads (needs full x), the causal
attention for those heads, and a row-parallel partial of the output
projection. Host sums the 8 partials and adds the (precomputable) bias
terms.

Device layout choices (all chosen to avoid on-device transposes):
  - x is passed host-transposed as xT[1024, 4096] so the PE (which
    contracts over the partition dim) can consume it directly.
  - everything on the matmul data path is bf16 (PSUM accumulation stays
    fp32): halves DMA traffic and SBUF footprint at the same PE rate.
  - q,k are produced transposed (qT/kT [128=2*64, 4096]) straight out of
    the qkv matmul; v is produced in natural [token, feat] layout via a
    PE transpose of the vT matmul result.
  - v is stored [v0 | ones | v1] (192 cols) so head0 reads cols 0:128
    and head1 reads cols 64:192 — the shared ones block makes the
    softmax denominator land bank-aligned with the numerator in the
    same PSUM tile for both heads (for head1 num/den are swapped).
  - attention scores are computed as S^T = k @ q^T in [tk, tq] blocks;
    on the diagonal superblock, columns < dlt*KB are fully masked and
    are skipped entirely (S, exp, and PV all trimmed); the triangle is
    one KB-wide sub-block masked by a 0/1 multiply on VectorE; the two
    heads' exps/masks are merged into single instructions via 3D APs.
  - emission is software-pipelined (A(n) / B(b,i) / C slices
    interleaved) so the PE-heavy qkv/proj phases overlap the
    ScalarE-heavy exp phase; proj (C) work is emitted eagerly as soon
    as a chunk's attns columns are final, and the tail C quanta rotate
    over spare PSUM banks with copies alternating DVE/ScalarE.
"""

import numpy as np
import ml_dtypes

import concourse.bass as bass
import concourse.tile as tile
from concourse import bacc, mybir
from concourse.bass_utils import run_bass_kernel_spmd

B, T, C = 2, 2048, 1024
H, D = 16, 64
TOK = B * T            # 4096
P = 128
NQ = 512               # q-chunk (moving free dim per head)
KB = 128               # k-block (PSUM partition dim)
KO = C // P            # 8 contraction subtiles
NCHUNK = TOK // NQ     # 8 token chunks
QC = T // NQ           # 4 q-chunks per batch
KBB = T // KB          # 16 k-blocks per batch
F32 = mybir.dt.float32
BF16 = mybir.dt.bfloat16
EXP = mybir.ActivationFunctionType.Exp

_CACHE = {}


def _build(debug_taps=False):
    nc = bacc.Bacc("TRN2", target_bir_lowering=False, debug=False, num_devices=8)
    marks = []
    _CACHE["marks"] = marks

    def mark(lbl):
        marks.append((nc.next_id(), lbl))

    # host pre-rearranged weights so every DMA is contiguous per partition
    xt_d = nc.dram_tensor("xt", [C, TOK], BF16, kind="ExternalInput").ap()
    wqk_d = nc.dram_tensor("wqk", [P, 2, KO, P], BF16, kind="ExternalInput").ap()
    bqk_d = nc.dram_tensor("bqk", [P, 2], F32, kind="ExternalInput").ap()
    wv_d = nc.dram_tensor("wv", [P, KO, P], BF16, kind="ExternalInput").ap()
    wproj_d = nc.dram_tensor("wproj", [P, C], BF16, kind="ExternalInput").ap()
    masks_d = nc.dram_tensor("masks", [P, 2, P], BF16, kind="ExternalInput").ap()
    ident_d = nc.dram_tensor("ident", [P, P], BF16, kind="ExternalInput").ap()
    y_d = nc.dram_tensor("y", [TOK, C], BF16, kind="ExternalOutput").ap()
    dbg = {}
    if debug_taps:
        dbg["qT"] = nc.dram_tensor("dbg_qT", [P, TOK], F32, kind="ExternalOutput").ap()
        dbg["kT"] = nc.dram_tensor("dbg_kT", [P, TOK], F32, kind="ExternalOutput").ap()
        dbg["v"] = nc.dram_tensor("dbg_v", [P, B * KBB, 3, D], F32, kind="ExternalOutput").ap()
        dbg["attns"] = nc.dram_tensor("dbg_attns", [P, TOK], F32, kind="ExternalOutput").ap()

    with tile.TileContext(nc) as tc:
        with tc.tile_pool(name="res", bufs=1) as res, \
             tc.tile_pool(name="xt", bufs=2) as xtp, \
             tc.tile_pool(name="pt", bufs=4) as ptp, \
             tc.tile_pool(name="vt", bufs=4) as vtp, \
             tc.tile_pool(name="ys", bufs=8) as ysp:
            # ---- resident tensors ----
            wqk_sb = res.tile([P, 2, KO, P], BF16, tag="wqk")
            bqk_sb = res.tile([P, 2], F32, tag="bqk")
            wv_sb = res.tile([P, KO, P], BF16, tag="wv")
            wproj_sb = res.tile([P, C], BF16, tag="wproj")
            masks_sb = res.tile([P, 2, P], BF16, tag="masks")
            ident_sb = res.tile([P, P], BF16, tag="ident")

            qT_sb = res.tile([P, TOK], BF16, tag="qT")
            kT_sb = res.tile([P, TOK], BF16, tag="kT")
            # [v0 | ones | v1] per k-block; h0 reads cols 0:128, h1 64:192
            v_sb = res.tile([P, B * KBB, 3, D], BF16, tag="v")
            attns_sb = res.tile([P, TOK], BF16, tag="attns")

            # shared denominator-replicator ones block
            nc.vector.memset(v_sb[:, :, 1, :], 1.0)

            # ---- filler machinery: A(qkv) and C(proj) work is split into
            # small PE quanta pumped between attention j-steps, so the PE
            # (in-order queue) always has ready work while ScalarE runs exp.
            from collections import deque
            fill_a = deque()          # A quanta (qkv) — deadline-ordered
            fill_c = deque()          # C quanta (proj) — deadline-free
            a_left_box = [0]
            rr = [0]

            def pump(k=1):
                # alternate A/C so proj matmuls+DMAs spread through the
                # whole timeline instead of piling up at the tail
                n = 0
                while n < k and (fill_a or fill_c):
                    rr[0] += 1
                    src = fill_c if (fill_c and (rr[0] % 2 == 0 or not fill_a)) \
                        else fill_a
                    if src is fill_a:
                        a_left_box[0] -= 1
                    src.popleft()()
                    n += 1

            def pump_a(k=1):
                n = 0
                while n < k and fill_a:
                    a_left_box[0] -= 1
                    fill_a.popleft()()
                    n += 1

            # ---- A quanta: qkv for one 512-token chunk ----
            xt_tiles = {}   # pair -> list of 8 [P, 2*NQ] tiles

            def emit_xt_dma(pair, halves=False):
                # halves=True loads only the first 512 tokens of each tile
                # (chunk 0) so the first qk matmuls aren't starved by the
                # 728ns full-tile DMA cadence; the second halves follow via
                # emit_xt_dma2.
                xts = []
                for k in range(KO):
                    xt = xtp.tile([P, 2 * NQ], BF16, tag=f"xt{k}", name="xt")
                    sl = slice(0, NQ) if halves else slice(0, 2 * NQ)
                    nc.sync.dma_start(
                        xt[:, sl], xt_d[k * P:(k + 1) * P,
                                        pair * 2 * NQ + sl.start:
                                        pair * 2 * NQ + sl.stop])
                    xts.append(xt)
                xt_tiles[pair] = xts

            def emit_xt_dma2(pair):
                for k, xt in enumerate(xt_tiles[pair]):
                    nc.sync.dma_start(
                        xt[:, NQ:], xt_d[k * P:(k + 1) * P,
                                         pair * 2 * NQ + NQ:
                                         (pair + 1) * 2 * NQ])

            def make_A_quanta(n, psF):
                st = {}
                pair, half = n // 2, n % 2
                hs = slice(half * NQ, (half + 1) * NQ)

                def q_dma():
                    mark(f"A{n}.dma")
                    emit_xt_dma(pair)

                def q_qk(m):
                    def f():
                        mark(f"A{n}.qk{m}")
                        xts = xt_tiles[pair]
                        pq = psF.tile([P, NQ], F32, tag=f"f{m}", name="pq")
                        for k in range(KO):
                            nc.tensor.matmul(
                                pq[:], wqk_sb[:, m, k, :], xts[k][:, hs],
                                start=(k == 0), stop=(k == KO - 1))
                        dst = qT_sb if m == 0 else kT_sb
                        nc.vector.tensor_scalar_add(
                            dst[:, n * NQ:(n + 1) * NQ], pq[:],
                            bqk_sb[:, m:m + 1])
                    return f

                def q_vT():
                    # vT[feat, tok] accumulated with wv stationary, staged to
                    # SBUF for the PE transpose back to [token, feat]
                    mark(f"A{n}.vT")
                    xts = xt_tiles[pair]
                    pvT = psF.tile([P, NQ], F32, tag="f0", name="pvT")
                    for k in range(KO):
                        nc.tensor.matmul(
                            pvT[:], wv_sb[:, k, :], xts[k][:, hs],
                            start=(k == 0), stop=(k == KO - 1))
                    vt = vtp.tile([P, NQ], BF16, tag="vt", name="vt")
                    nc.vector.tensor_copy(vt[:], pvT[:])
                    st["vt"] = vt

                def q_tp(m2):
                    # PE-transpose one [128,128] block of vT back to natural
                    # [token, feat] layout; single copy scatters v0/v1 around
                    # the shared ones block
                    def f():
                        mark(f"A{n}.tp{m2}")
                        tp = psF.tile([P, P], BF16, tag=f"f{m2 % 2}", name="tp")
                        nc.tensor.transpose(
                            tp[:], st["vt"][:, m2 * P:(m2 + 1) * P], ident_sb[:])
                        kb = n * 4 + m2
                        nc.vector.tensor_copy(
                            v_sb[:, kb, 0:3:2, :],
                            tp[:].rearrange("p (h c) -> p h c", h=2))
                    return f

                qs = [q_dma] if (half == 0 and pair != 0) else []
                return qs + [q_qk(0), q_qk(1), q_vT,
                             q_tp(0), q_tp(1), q_tp(2), q_tp(3)]

            # ---- C quanta: output projection for one (m-block, half).
            # In tail mode (attention finished) the PSUM rotation widens
            # over the freed psS banks and copies alternate DVE/ScalarE.
            ys_tiles = {}
            c_state = {"n": 0, "alt": False, "tail": False}

            def make_C_quantum(m, n2):
                def f():
                    mark(f"C.m{m}.{n2}")
                    if n2 == 0:
                        ys_tiles[m] = ysp.tile([P, C], BF16, tag="ys", name="ys")
                    ys = ys_tiles[m]
                    cn = c_state["n"]
                    c_state["n"] += 1
                    if c_state["tail"]:
                        pool, tag = (psF, f"f{cn % 2}") \
                            if cn % 4 < 2 else (psS_g, "s")
                    else:
                        pool, tag = psF, f"f{cn % 2}"
                    # once ScalarE's exp backlog thins, shift copies onto it:
                    # 1-in-3 during batch-1 windows, 1-in-2 at the tail
                    if c_state["alt"]:
                        eng = "act" if cn % 2 else "dve"
                    elif c_state.get("b1"):
                        eng = "act" if cn % 3 == 2 else "dve"
                    else:
                        eng = "dve"
                    py = pool.tile([P, NQ], F32, tag=tag, name="py")
                    nc.tensor.matmul(
                        py[:], attns_sb[:, m * P:(m + 1) * P],
                        wproj_sb[:, n2 * NQ:(n2 + 1) * NQ],
                        start=True, stop=True)
                    if eng == "act":
                        nc.scalar.copy(ys[:, n2 * NQ:(n2 + 1) * NQ], py[:])
                    else:
                        nc.vector.tensor_copy(ys[:, n2 * NQ:(n2 + 1) * NQ], py[:])
                    if n2 == 1:
                        nc.sync.dma_start(y_d[m * P:(m + 1) * P, :], ys[:])
                        del ys_tiles[m]
                return f

            # ---- stage B: attention for batch b, token window [q0, q0+qw) ----
            js_left_box = [80]  # total j-steps over all B windows

            def emit_B(b, q0, qw, quiet=False):
                nq0 = b * T + q0
                jmax = (q0 + qw) // KB
                jdiag = q0 // KB   # j >= jdiag overlaps the q window
                psS, psO = psS_g, psO_g
                po = [psO.tile([P, NQ], F32, tag=f"o{h}", name=f"po{h}")
                      for h in range(2)]
                s_tiles = {}

                def emit_s(j):
                    # diagonal superblock: q-columns < j*KB - q0 are fully
                    # masked — skip them in S, exp and PV alike
                    c0 = max(0, j * KB - q0)
                    s = psS.tile([P, 2, NQ], F32, tag="s", name="s")
                    for h in range(2):
                        nc.tensor.matmul(
                            s[:, h, c0:qw],
                            kT_sb[h * D:(h + 1) * D,
                                  b * T + j * KB: b * T + (j + 1) * KB],
                            qT_sb[h * D:(h + 1) * D, nq0 + c0:nq0 + qw],
                            start=True, stop=True)
                    s_tiles[j] = (s, c0)

                emit_s(0)
                budget0 = (len(fill_a) + len(fill_c)) * jmax // js_left_box[0]
                js_left_box[0] -= jmax
                taken = 0
                for j in range(jmax):
                    mark(f"B{b}.{q0 // NQ}.j{j}")
                    if j + 1 < jmax:
                        emit_s(j + 1)
                    s, c0 = s_tiles.pop(j)
                    pt = ptp.tile([P, 2, NQ], BF16, tag="pt", name="pt")
                    if j >= jdiag:
                        # both heads' exp in one instruction (3D AP), then
                        # one merged triangle-mask multiply
                        nc.scalar.activation(pt[:, :, c0:qw], s[:, :, c0:qw], EXP)
                        nc.vector.tensor_mul(
                            pt[:, :, c0:c0 + KB], pt[:, :, c0:c0 + KB],
                            masks_sb[:])
                    else:
                        nc.scalar.activation(pt[:, :, 0:qw], s[:, :, 0:qw], EXP)
                    # cap filler per j-step so PV(j) never queues behind a
                    # long filler burst on the in-order PE queue; on the last
                    # step pump only after the norm ops are queued, so the
                    # chunk's normalization isn't stuck behind filler copies
                    if j + 1 < jmax and not quiet:
                        want = min(budget0 * (j + 1) // jmax, 4 * (j + 1))
                        if want > taken:
                            pump(want - taken)
                            taken = want
                    for h in range(2):
                        nc.tensor.matmul(
                            po[h][:, c0:qw], v_sb[:, b * KBB + j, h:h + 2, :],
                            pt[:, h, c0:qw],
                            start=(j == 0), stop=(j == jmax - 1))
                        if j == jmax - 1:
                            # normalize this head immediately: its recip
                            # runs on DVE while PE starts the other head.
                            # head1's [ones|v1] layout swaps num/den rows.
                            nd = D if h == 0 else 0
                            rc = ptp.tile([D, NQ], F32, tag="rc", name="rc")
                            nc.vector.reciprocal(rc[:, 0:qw], po[h][nd:nd + D, 0:qw])
                            nc.vector.tensor_mul(
                                attns_sb[h * D:(h + 1) * D, nq0:nq0 + qw],
                                po[h][D - nd:2 * D - nd, 0:qw], rc[:, 0:qw])
                if budget0 > taken and not quiet:
                    pump(budget0 - taken)

            # ---- interleaved emission ----
            with tc.tile_pool(name="psF", bufs=1, space="PSUM") as psF, \
                 tc.tile_pool(name="psS", bufs=2, space="PSUM") as psS_g, \
                 tc.tile_pool(name="psO", bufs=1, space="PSUM") as psO_g:
                # prologue DMAs, ordered so A0.qk0 can start earliest:
                # first the k=0..1 slice of wqk[m=0] and the k-ascending xt
                # tiles, so the first accumulation matmuls fire while the
                # rest of the weights stream in behind them.
                nc.sync.dma_start(wqk_sb[:, 0], wqk_d[:, 0])
                emit_xt_dma(0)
                nc.sync.dma_start(bqk_sb[:], bqk_d[:])
                nc.sync.dma_start(wqk_sb[:, 1], wqk_d[:, 1])
                nc.sync.dma_start(wv_sb[:], wv_d[:])
                nc.sync.dma_start(ident_sb[:], ident_d[:])
                nc.sync.dma_start(masks_sb[:], masks_d[:])
                nc.sync.dma_start(wproj_sb[:], wproj_d[:])

                a_total = 0
                a_prefix = [0]
                for n in range(NCHUNK):
                    qs = make_A_quanta(n, psF)
                    fill_a.extend(qs)
                    a_total += len(qs)
                    a_prefix.append(a_total)
                a_left_box[0] = a_total

                # batch-0 in order; batch-1 big windows first, then the
                # first 512 tokens as two 256-token sub-windows so the final
                # proj+DMA tail overlaps the last sub-window's attention
                windows = [(0, q0, NQ) for q0 in range(0, T, NQ)]
                windows += [(1, q0, NQ) for q0 in (NQ, 2 * NQ, 3 * NQ)]
                windows += [(1, 0, NQ)]
                for wi, (b, q0, qw) in enumerate(windows):
                    last = wi == len(windows) - 1
                    if b == 1:
                        c_state["b1"] = True
                    if last:
                        c_state["alt"] = True
                    # A chunks needed by this window must be done first
                    need = a_total - a_prefix[(b * T + q0 + qw + NQ - 1) // NQ]
                    if a_left_box[0] > need:
                        pump_a(a_left_box[0] - need)
                    emit_B(b, q0, qw, quiet=last)
                    for mm in range(qw // P):
                        m = (b * T + q0) // P + mm
                        for n2 in range(2):
                            fill_c.append(make_C_quantum(m, n2))
                # trailing drain: attention is done, so the C rotation can
                # widen over the freed psS banks and use both copy engines
                c_state["tail"] = True
                pump_a(len(fill_a))
                while fill_c:
                    fill_c.popleft()()

            if debug_taps:
                dbg  # debug taps disabled in bf16 build

    nc.compile()
    return nc


def _host_prep(x, W_qkv, b_qkv, W_proj, b_proj):
    bf16 = ml_dtypes.bfloat16
    x = np.ascontiguousarray(np.asarray(x, dtype=np.float32))
    W_qkv = np.asarray(W_qkv, dtype=np.float32)
    b_qkv = np.asarray(b_qkv, dtype=np.float32)
    W_proj = np.asarray(W_proj, dtype=np.float32)
    b_proj = np.asarray(b_proj, dtype=np.float32)

    xT = np.ascontiguousarray(x.reshape(TOK, C).T.astype(bf16))  # [1024, 4096]
    scale = np.float32(1.0 / np.sqrt(D))

    masks = np.triu(np.ones((P, P), dtype=np.float32))  # [tk, tq]: tq >= tk
    masks2 = np.ascontiguousarray(
        np.stack([masks, masks], axis=1).astype(bf16))  # [P, 2, P]
    ident = np.ascontiguousarray(np.eye(P, dtype=np.float32).astype(bf16))

    in_maps = []
    for c in range(8):
        s0, s1 = c * P, (c + 1) * P
        wq = W_qkv[:, s0:s1] * scale
        wk = W_qkv[:, C + s0:C + s1]
        wv = W_qkv[:, 2 * C + s0:2 * C + s1]
        bq = b_qkv[s0:s1] * scale
        bk = b_qkv[C + s0:C + s1]
        # wqk host-rearranged to [p, m, ko, pcol] so DMAs are contiguous
        wqk = np.stack([wq, wk], axis=0)                 # [2, 1024, 128]
        wqk = wqk.reshape(2, KO, P, P).transpose(2, 0, 1, 3)  # [p, m, ko, pc]
        in_maps.append({
            "xt": xT,
            "wqk": np.ascontiguousarray(wqk.astype(bf16)),
            "bqk": np.ascontiguousarray(np.stack([bq, bk], axis=1)),
            "wv": np.ascontiguousarray(
                wv.reshape(KO, P, P).transpose(1, 0, 2).astype(bf16)),
            "wproj": np.ascontiguousarray(W_proj[s0:s1, :].astype(bf16)),
            "masks": masks2,
            "ident": ident,
        })
    # constant bias terms folded on host:
    #   out_proj bias + (v-bias row) @ W_proj  (v bias passes through softmax)
    ybias = b_qkv[2 * C:3 * C] @ W_proj + b_proj  # [1024]
    return in_maps, ybias


def kernel(x, W_qkv, b_qkv, W_proj, b_proj):
    if "nc" not in _CACHE:
        _CACHE["nc"] = _build()
    nc = _CACHE["nc"]
    in_maps, ybias = _host_prep(x, W_qkv, b_qkv, W_proj, b_proj)
    try:
        res = run_bass_kernel_spmd(nc, in_maps, core_ids=list(range(8)))
    except Exception:
        # transient device errors (NRT_EXEC_UNIT_UNRECOVERABLE) heal on retry
        res = run_bass_kernel_spmd(nc, in_maps, core_ids=list(range(8)))
    y = np.zeros((TOK, C), dtype=np.float32)
    for c in range(8):
        y += np.asarray(res.results[c]["y"]).astype(np.float32)
    y += ybias[None, :].astype(np.float32)
    return y.reshape(B, T, C)


# revision 37
# speedup vs baseline: 1.0013x; 1.0013x over previous
"""Causal multi-head attention on 8 Trainium2 NeuronCores (Bass/Tile).

Problem (hardcoded): x[2,2048,1024], W_qkv[1024,3072], b_qkv[3072],
W_proj[1024,1024], b_proj[1024]; 16 heads, head_dim 64, causal softmax.

Sharding: tensor-parallel over heads — core c owns heads (2c, 2c+1).
Each core computes qkv for its 2 heads (needs full x), the causal
attention for those heads, and a row-parallel partial of the output
projection. Host sums the 8 partials and adds the (precomputable) bias
terms.

Device layout choices (all chosen to avoid on-device transposes):
  - x is passed host-transposed as xT[1024, 4096] so the PE (which
    contracts over the partition dim) can consume it directly.
  - everything on the matmul data path is bf16 (PSUM accumulation stays
    fp32): halves DMA traffic and SBUF footprint at the same PE rate.
  - q,k are produced transposed (qT/kT [128=2*64, 4096]) straight out of
    the qkv matmul; v is produced in natural [token, feat] layout via a
    PE transpose of the vT matmul result.
  - v is stored [v0 | ones | v1] (192 cols) so head0 reads cols 0:128
    and head1 reads cols 64:192 — the shared ones block makes the
    softmax denominator land bank-aligned with the numerator in the
    same PSUM tile for both heads (for head1 num/den are swapped).
  - attention scores are computed as S^T = k @ q^T in [tk, tq] blocks;
    on the diagonal superblock, columns < dlt*KB are fully masked and
    are skipped entirely (S, exp, and PV all trimmed); the triangle is
    one KB-wide sub-block masked by a 0/1 multiply on VectorE; the two
    heads' exps/masks are merged into single instructions via 3D APs.
  - emission is software-pipelined (A(n) / B(b,i) / C slices
    interleaved) so the PE-heavy qkv/proj phases overlap the
    ScalarE-heavy exp phase; proj (C) work is emitted eagerly as soon
    as a chunk's attns columns are final, and the tail C quanta rotate
    over spare PSUM banks with copies alternating DVE/ScalarE.
"""

import numpy as np
import ml_dtypes

import concourse.bass as bass
import concourse.tile as tile
from concourse import bacc, mybir
from concourse.bass_utils import run_bass_kernel_spmd

B, T, C = 2, 2048, 1024
H, D = 16, 64
TOK = B * T            # 4096
P = 128
NQ = 512               # q-chunk (moving free dim per head)
KB = 128               # k-block (PSUM partition dim)
KO = C // P            # 8 contraction subtiles
NCHUNK = TOK // NQ     # 8 token chunks
QC = T // NQ           # 4 q-chunks per batch
KBB = T // KB          # 16 k-blocks per batch
F32 = mybir.dt.float32
BF16 = mybir.dt.bfloat16
EXP = mybir.ActivationFunctionType.Exp

_CACHE = {}


def _build(debug_taps=False):
    nc = bacc.Bacc("TRN2", target_bir_lowering=False, debug=False, num_devices=8)
    marks = []
    _CACHE["marks"] = marks

    def mark(lbl):
        marks.append((nc.next_id(), lbl))

    # host pre-rearranged weights so every DMA is contiguous per partition
    xt_d = nc.dram_tensor("xt", [C, TOK], BF16, kind="ExternalInput").ap()
    wqk_d = nc.dram_tensor("wqk", [P, 2, KO, P], BF16, kind="ExternalInput").ap()
    bqk_d = nc.dram_tensor("bqk", [P, 2], F32, kind="ExternalInput").ap()
    wv_d = nc.dram_tensor("wv", [P, KO, P], BF16, kind="ExternalInput").ap()
    wproj_d = nc.dram_tensor("wproj", [P, C], BF16, kind="ExternalInput").ap()
    masks_d = nc.dram_tensor("masks", [P, 2, P], BF16, kind="ExternalInput").ap()
    ident_d = nc.dram_tensor("ident", [P, P], BF16, kind="ExternalInput").ap()
    y_d = nc.dram_tensor("y", [TOK, C], BF16, kind="ExternalOutput").ap()
    dbg = {}
    if debug_taps:
        dbg["qT"] = nc.dram_tensor("dbg_qT", [P, TOK], F32, kind="ExternalOutput").ap()
        dbg["kT"] = nc.dram_tensor("dbg_kT", [P, TOK], F32, kind="ExternalOutput").ap()
        dbg["v"] = nc.dram_tensor("dbg_v", [P, B * KBB, 3, D], F32, kind="ExternalOutput").ap()
        dbg["attns"] = nc.dram_tensor("dbg_attns", [P, TOK], F32, kind="ExternalOutput").ap()

    with tile.TileContext(nc) as tc:
        with tc.tile_pool(name="res", bufs=1) as res, \
             tc.tile_pool(name="xt", bufs=2) as xtp, \
             tc.tile_pool(name="pt", bufs=4) as ptp, \
             tc.tile_pool(name="vt", bufs=4) as vtp, \
             tc.tile_pool(name="ys", bufs=8) as ysp:
            # ---- resident tensors ----
            wqk_sb = res.tile([P, 2, KO, P], BF16, tag="wqk")
            bqk_sb = res.tile([P, 2], F32, tag="bqk")
            wv_sb = res.tile([P, KO, P], BF16, tag="wv")
            wproj_sb = res.tile([P, C], BF16, tag="wproj")
            masks_sb = res.tile([P, 2, P], BF16, tag="masks")
            ident_sb = res.tile([P, P], BF16, tag="ident")

            qT_sb = res.tile([P, TOK], BF16, tag="qT")
            kT_sb = res.tile([P, TOK], BF16, tag="kT")
            # [v0 | ones | v1] per k-block; h0 reads cols 0:128, h1 64:192
            v_sb = res.tile([P, B * KBB, 3, D], BF16, tag="v")
            attns_sb = res.tile([P, TOK], BF16, tag="attns")

            # shared denominator-replicator ones block
            nc.vector.memset(v_sb[:, :, 1, :], 1.0)

            # ---- filler machinery: A(qkv) and C(proj) work is split into
            # small PE quanta pumped between attention j-steps, so the PE
            # (in-order queue) always has ready work while ScalarE runs exp.
            from collections import deque
            fill_a = deque()          # A quanta (qkv) — deadline-ordered
            fill_c = deque()          # C quanta (proj) — deadline-free
            a_left_box = [0]
            rr = [0]

            def pump(k=1):
                # alternate A/C so proj matmuls+DMAs spread through the
                # whole timeline instead of piling up at the tail
                n = 0
                while n < k and (fill_a or fill_c):
                    rr[0] += 1
                    src = fill_c if (fill_c and (rr[0] % 2 == 0 or not fill_a)) \
                        else fill_a
                    if src is fill_a:
                        a_left_box[0] -= 1
                    src.popleft()()
                    n += 1

            def pump_a(k=1):
                n = 0
                while n < k and fill_a:
                    a_left_box[0] -= 1
                    fill_a.popleft()()
                    n += 1

            # ---- A quanta: qkv for one 512-token chunk ----
            xt_tiles = {}   # pair -> list of 8 [P, 2*NQ] tiles

            def emit_xt_dma(pair, halves=False):
                # halves=True loads only the first 512 tokens of each tile
                # (chunk 0) so the first qk matmuls aren't starved by the
                # 728ns full-tile DMA cadence; the second halves follow via
                # emit_xt_dma2.
                xts = []
                for k in range(KO):
                    xt = xtp.tile([P, 2 * NQ], BF16, tag=f"xt{k}", name="xt")
                    sl = slice(0, NQ) if halves else slice(0, 2 * NQ)
                    nc.sync.dma_start(
                        xt[:, sl], xt_d[k * P:(k + 1) * P,
                                        pair * 2 * NQ + sl.start:
                                        pair * 2 * NQ + sl.stop])
                    xts.append(xt)
                xt_tiles[pair] = xts

            def emit_xt_dma2(pair):
                for k, xt in enumerate(xt_tiles[pair]):
                    nc.sync.dma_start(
                        xt[:, NQ:], xt_d[k * P:(k + 1) * P,
                                         pair * 2 * NQ + NQ:
                                         (pair + 1) * 2 * NQ])

            def make_A_quanta(n, psF):
                st = {}
                pair, half = n // 2, n % 2
                hs = slice(half * NQ, (half + 1) * NQ)

                def q_dma():
                    mark(f"A{n}.dma")
                    emit_xt_dma(pair)

                def q_qk(m):
                    def f():
                        mark(f"A{n}.qk{m}")
                        xts = xt_tiles[pair]
                        pq = psF.tile([P, NQ], F32, tag=f"f{m}", name="pq")
                        for k in range(KO):
                            nc.tensor.matmul(
                                pq[:], wqk_sb[:, m, k, :], xts[k][:, hs],
                                start=(k == 0), stop=(k == KO - 1))
                        dst = qT_sb if m == 0 else kT_sb
                        nc.vector.tensor_scalar_add(
                            dst[:, n * NQ:(n + 1) * NQ], pq[:],
                            bqk_sb[:, m:m + 1])
                    return f

                def q_vT():
                    # vT[feat, tok] accumulated with wv stationary, staged to
                    # SBUF for the PE transpose back to [token, feat]
                    mark(f"A{n}.vT")
                    xts = xt_tiles[pair]
                    pvT = psF.tile([P, NQ], F32, tag="f0", name="pvT")
                    for k in range(KO):
                        nc.tensor.matmul(
                            pvT[:], wv_sb[:, k, :], xts[k][:, hs],
                            start=(k == 0), stop=(k == KO - 1))
                    vt = vtp.tile([P, NQ], BF16, tag="vt", name="vt")
                    nc.vector.tensor_copy(vt[:], pvT[:])
                    st["vt"] = vt

                def q_tp(m2):
                    # PE-transpose one [128,128] block of vT back to natural
                    # [token, feat] layout; single copy scatters v0/v1 around
                    # the shared ones block
                    def f():
                        mark(f"A{n}.tp{m2}")
                        tp = psF.tile([P, P], BF16, tag=f"f{m2 % 2}", name="tp")
                        nc.tensor.transpose(
                            tp[:], st["vt"][:, m2 * P:(m2 + 1) * P], ident_sb[:])
                        kb = n * 4 + m2
                        nc.vector.tensor_copy(
                            v_sb[:, kb, 0:3:2, :],
                            tp[:].rearrange("p (h c) -> p h c", h=2))
                    return f

                qs = [q_dma] if (half == 0 and pair != 0) else []
                return qs + [q_qk(0), q_qk(1), q_vT,
                             q_tp(0), q_tp(1), q_tp(2), q_tp(3)]

            # ---- C quanta: output projection for one (m-block, half).
            # In tail mode (attention finished) the PSUM rotation widens
            # over the freed psS banks and copies alternate DVE/ScalarE.
            ys_tiles = {}
            c_state = {"n": 0, "alt": False, "tail": False}

            def make_C_quantum(m, n2):
                def f():
                    mark(f"C.m{m}.{n2}")
                    if n2 == 0:
                        ys_tiles[m] = ysp.tile([P, C], BF16, tag="ys", name="ys")
                    ys = ys_tiles[m]
                    cn = c_state["n"]
                    c_state["n"] += 1
                    if c_state["tail"]:
                        pool, tag = (psF, f"f{cn % 2}") \
                            if cn % 4 < 2 else (psS_g, "s")
                    else:
                        pool, tag = psF, f"f{cn % 2}"
                    # once ScalarE's exp backlog thins (last window onward),
                    # alternate copies over both engines so the in-order DVE
                    # queue can't stall the attention chain
                    eng = "act" if (c_state["alt"] and cn % 2) else "dve"
                    py = pool.tile([P, NQ], F32, tag=tag, name="py")
                    nc.tensor.matmul(
                        py[:], attns_sb[:, m * P:(m + 1) * P],
                        wproj_sb[:, n2 * NQ:(n2 + 1) * NQ],
                        start=True, stop=True)
                    if eng == "act":
                        nc.scalar.copy(ys[:, n2 * NQ:(n2 + 1) * NQ], py[:])
                    else:
                        nc.vector.tensor_copy(ys[:, n2 * NQ:(n2 + 1) * NQ], py[:])
                    if n2 == 1:
                        nc.sync.dma_start(y_d[m * P:(m + 1) * P, :], ys[:])
                        del ys_tiles[m]
                return f

            # ---- stage B: attention for batch b, token window [q0, q0+qw) ----
            js_left_box = [80]  # total j-steps over all B windows

            def emit_B(b, q0, qw, quiet=False):
                nq0 = b * T + q0
                jmax = (q0 + qw) // KB
                jdiag = q0 // KB   # j >= jdiag overlaps the q window
                psS, psO = psS_g, psO_g
                po = [psO.tile([P, NQ], F32, tag=f"o{h}", name=f"po{h}")
                      for h in range(2)]
                s_tiles = {}

                def emit_s(j):
                    # diagonal superblock: q-columns < j*KB - q0 are fully
                    # masked — skip them in S, exp and PV alike
                    c0 = max(0, j * KB - q0)
                    s = psS.tile([P, 2, NQ], F32, tag="s", name="s")
                    for h in range(2):
                        nc.tensor.matmul(
                            s[:, h, c0:qw],
                            kT_sb[h * D:(h + 1) * D,
                                  b * T + j * KB: b * T + (j + 1) * KB],
                            qT_sb[h * D:(h + 1) * D, nq0 + c0:nq0 + qw],
                            start=True, stop=True)
                    s_tiles[j] = (s, c0)

                emit_s(0)
                budget0 = (len(fill_a) + len(fill_c)) * jmax // js_left_box[0]
                js_left_box[0] -= jmax
                taken = 0
                for j in range(jmax):
                    mark(f"B{b}.{q0 // NQ}.j{j}")
                    if j + 1 < jmax:
                        emit_s(j + 1)
                    s, c0 = s_tiles.pop(j)
                    pt = ptp.tile([P, 2, NQ], BF16, tag="pt", name="pt")
                    if j >= jdiag:
                        # both heads' exp in one instruction (3D AP), then
                        # one merged triangle-mask multiply
                        nc.scalar.activation(pt[:, :, c0:qw], s[:, :, c0:qw], EXP)
                        nc.vector.tensor_mul(
                            pt[:, :, c0:c0 + KB], pt[:, :, c0:c0 + KB],
                            masks_sb[:])
                    else:
                        nc.scalar.activation(pt[:, :, 0:qw], s[:, :, 0:qw], EXP)
                    # cap filler per j-step so PV(j) never queues behind a
                    # long filler burst on the in-order PE queue; on the last
                    # step pump only after the norm ops are queued, so the
                    # chunk's normalization isn't stuck behind filler copies
                    if j + 1 < jmax and not quiet:
                        want = min(budget0 * (j + 1) // jmax, 3 * (j + 1))
                        if want > taken:
                            pump(want - taken)
                            taken = want
                    for h in range(2):
                        nc.tensor.matmul(
                            po[h][:, c0:qw], v_sb[:, b * KBB + j, h:h + 2, :],
                            pt[:, h, c0:qw],
                            start=(j == 0), stop=(j == jmax - 1))
                        if j == jmax - 1:
                            # normalize this head immediately: its recip
                            # runs on DVE while PE starts the other head.
                            # head1's [ones|v1] layout swaps num/den rows.
                            nd = D if h == 0 else 0
                            rc = ptp.tile([D, NQ], F32, tag="rc", name="rc")
                            nc.vector.reciprocal(rc[:, 0:qw], po[h][nd:nd + D, 0:qw])
                            nc.vector.tensor_mul(
                                attns_sb[h * D:(h + 1) * D, nq0:nq0 + qw],
                                po[h][D - nd:2 * D - nd, 0:qw], rc[:, 0:qw])
                if budget0 > taken and not quiet:
                    pump(budget0 - taken)

            # ---- interleaved emission ----
            with tc.tile_pool(name="psF", bufs=1, space="PSUM") as psF, \
                 tc.tile_pool(name="psS", bufs=2, space="PSUM") as psS_g, \
                 tc.tile_pool(name="psO", bufs=1, space="PSUM") as psO_g:
                # prologue DMAs, ordered so A0.qk0 can start earliest:
                # first the k=0..1 slice of wqk[m=0] and the k-ascending xt
                # tiles, so the first accumulation matmuls fire while the
                # rest of the weights stream in behind them.
                nc.sync.dma_start(wqk_sb[:, 0], wqk_d[:, 0])
                emit_xt_dma(0)
                nc.sync.dma_start(bqk_sb[:], bqk_d[:])
                nc.sync.dma_start(wqk_sb[:, 1], wqk_d[:, 1])
                nc.sync.dma_start(wv_sb[:], wv_d[:])
                nc.sync.dma_start(ident_sb[:], ident_d[:])
                nc.sync.dma_start(masks_sb[:], masks_d[:])
                nc.sync.dma_start(wproj_sb[:], wproj_d[:])

                a_total = 0
                a_prefix = [0]
                for n in range(NCHUNK):
                    qs = make_A_quanta(n, psF)
                    fill_a.extend(qs)
                    a_total += len(qs)
                    a_prefix.append(a_total)
                a_left_box[0] = a_total

                # batch-0 in order; batch-1 big windows first, then the
                # first 512 tokens as two 256-token sub-windows so the final
                # proj+DMA tail overlaps the last sub-window's attention
                windows = [(0, q0, NQ) for q0 in range(0, T, NQ)]
                windows += [(1, q0, NQ) for q0 in (NQ, 2 * NQ, 3 * NQ)]
                windows += [(1, 0, NQ)]
                for wi, (b, q0, qw) in enumerate(windows):
                    last = wi == len(windows) - 1
                    if last:
                        c_state["alt"] = True
                    # A chunks needed by this window must be done first
                    need = a_total - a_prefix[(b * T + q0 + qw + NQ - 1) // NQ]
                    if a_left_box[0] > need:
                        pump_a(a_left_box[0] - need)
                    emit_B(b, q0, qw)
                    for mm in range(qw // P):
                        m = (b * T + q0) // P + mm
                        for n2 in range(2):
                            fill_c.append(make_C_quantum(m, n2))
                # trailing drain: attention is done, so the C rotation can
                # widen over the freed psS banks and use both copy engines
                c_state["tail"] = True
                pump_a(len(fill_a))
                while fill_c:
                    fill_c.popleft()()

            if debug_taps:
                dbg  # debug taps disabled in bf16 build

    nc.compile()
    return nc


def _host_prep(x, W_qkv, b_qkv, W_proj, b_proj):
    bf16 = ml_dtypes.bfloat16
    x = np.ascontiguousarray(np.asarray(x, dtype=np.float32))
    W_qkv = np.asarray(W_qkv, dtype=np.float32)
    b_qkv = np.asarray(b_qkv, dtype=np.float32)
    W_proj = np.asarray(W_proj, dtype=np.float32)
    b_proj = np.asarray(b_proj, dtype=np.float32)

    xT = np.ascontiguousarray(x.reshape(TOK, C).T.astype(bf16))  # [1024, 4096]
    scale = np.float32(1.0 / np.sqrt(D))

    masks = np.triu(np.ones((P, P), dtype=np.float32))  # [tk, tq]: tq >= tk
    masks2 = np.ascontiguousarray(
        np.stack([masks, masks], axis=1).astype(bf16))  # [P, 2, P]
    ident = np.ascontiguousarray(np.eye(P, dtype=np.float32).astype(bf16))

    in_maps = []
    for c in range(8):
        s0, s1 = c * P, (c + 1) * P
        wq = W_qkv[:, s0:s1] * scale
        wk = W_qkv[:, C + s0:C + s1]
        wv = W_qkv[:, 2 * C + s0:2 * C + s1]
        bq = b_qkv[s0:s1] * scale
        bk = b_qkv[C + s0:C + s1]
        # wqk host-rearranged to [p, m, ko, pcol] so DMAs are contiguous
        wqk = np.stack([wq, wk], axis=0)                 # [2, 1024, 128]
        wqk = wqk.reshape(2, KO, P, P).transpose(2, 0, 1, 3)  # [p, m, ko, pc]
        in_maps.append({
            "xt": xT,
            "wqk": np.ascontiguousarray(wqk.astype(bf16)),
            "bqk": np.ascontiguousarray(np.stack([bq, bk], axis=1)),
            "wv": np.ascontiguousarray(
                wv.reshape(KO, P, P).transpose(1, 0, 2).astype(bf16)),
            "wproj": np.ascontiguousarray(W_proj[s0:s1, :].astype(bf16)),
            "masks": masks2,
            "ident": ident,
        })
    # constant bias terms folded on host:
    #   out_proj bias + (v-bias row) @ W_proj  (v bias passes through softmax)
    ybias = b_qkv[2 * C:3 * C] @ W_proj + b_proj  # [1024]
    return in_maps, ybias


def kernel(x, W_qkv, b_qkv, W_proj, b_proj):
    if "nc" not in _CACHE:
        _CACHE["nc"] = _build()
    nc = _CACHE["nc"]
    in_maps, ybias = _host_prep(x, W_qkv, b_qkv, W_proj, b_proj)
    try:
        res = run_bass_kernel_spmd(nc, in_maps, core_ids=list(range(8)))
    except Exception:
        # transient device errors (NRT_EXEC_UNIT_UNRECOVERABLE) heal on retry
        res = run_bass_kernel_spmd(nc, in_maps, core_ids=list(range(8)))
    y = np.zeros((TOK, C), dtype=np.float32)
    for c in range(8):
        y += np.asarray(res.results[c]["y"]).astype(np.float32)
    y += ybias[None, :].astype(np.float32)
    return y.reshape(B, T, C)


# revision 38
# speedup vs baseline: 1.0056x; 1.0043x over previous
"""Causal multi-head attention on 8 Trainium2 NeuronCores (Bass/Tile).

Problem (hardcoded): x[2,2048,1024], W_qkv[1024,3072], b_qkv[3072],
W_proj[1024,1024], b_proj[1024]; 16 heads, head_dim 64, causal softmax.

Sharding: tensor-parallel over heads — core c owns heads (2c, 2c+1).
Each core computes qkv for its 2 heads (needs full x), the causal
attention for those heads, and a row-parallel partial of the output
projection. Host sums the 8 partials and adds the (precomputable) bias
terms.

Device layout choices (all chosen to avoid on-device transposes):
  - x is passed host-transposed as xT[1024, 4096] so the PE (which
    contracts over the partition dim) can consume it directly.
  - everything on the matmul data path is bf16 (PSUM accumulation stays
    fp32): halves DMA traffic and SBUF footprint at the same PE rate.
  - q,k are produced transposed (qT/kT [128=2*64, 4096]) straight out of
    the qkv matmul; v is produced in natural [token, feat] layout via a
    PE transpose of the vT matmul result.
  - v is stored [v0 | ones | v1] (192 cols) so head0 reads cols 0:128
    and head1 reads cols 64:192 — the shared ones block makes the
    softmax denominator land bank-aligned with the numerator in the
    same PSUM tile for both heads (for head1 num/den are swapped).
  - attention scores are computed as S^T = k @ q^T in [tk, tq] blocks;
    on the diagonal superblock, columns < dlt*KB are fully masked and
    are skipped entirely (S, exp, and PV all trimmed); the triangle is
    one KB-wide sub-block masked by a 0/1 multiply on VectorE; the two
    heads' exps/masks are merged into single instructions via 3D APs.
  - emission is software-pipelined (A(n) / B(b,i) / C slices
    interleaved) so the PE-heavy qkv/proj phases overlap the
    ScalarE-heavy exp phase; proj (C) work is emitted eagerly as soon
    as a chunk's attns columns are final, and the tail C quanta rotate
    over spare PSUM banks with copies alternating DVE/ScalarE.
"""

import numpy as np
import ml_dtypes

import concourse.bass as bass
import concourse.tile as tile
from concourse import bacc, mybir
from concourse.bass_utils import run_bass_kernel_spmd

B, T, C = 2, 2048, 1024
H, D = 16, 64
TOK = B * T            # 4096
P = 128
NQ = 512               # q-chunk (moving free dim per head)
KB = 128               # k-block (PSUM partition dim)
KO = C // P            # 8 contraction subtiles
NCHUNK = TOK // NQ     # 8 token chunks
QC = T // NQ           # 4 q-chunks per batch
KBB = T // KB          # 16 k-blocks per batch
F32 = mybir.dt.float32
BF16 = mybir.dt.bfloat16
EXP = mybir.ActivationFunctionType.Exp

_CACHE = {}


def _build(debug_taps=False):
    nc = bacc.Bacc("TRN2", target_bir_lowering=False, debug=False, num_devices=8)
    marks = []
    _CACHE["marks"] = marks

    def mark(lbl):
        marks.append((nc.next_id(), lbl))

    # host pre-rearranged weights so every DMA is contiguous per partition
    xt_d = nc.dram_tensor("xt", [C, TOK], BF16, kind="ExternalInput").ap()
    wqk_d = nc.dram_tensor("wqk", [P, 2, KO, P], BF16, kind="ExternalInput").ap()
    bqk_d = nc.dram_tensor("bqk", [P, 2], F32, kind="ExternalInput").ap()
    wv_d = nc.dram_tensor("wv", [P, KO, P], BF16, kind="ExternalInput").ap()
    wproj_d = nc.dram_tensor("wproj", [P, C], BF16, kind="ExternalInput").ap()
    masks_d = nc.dram_tensor("masks", [P, 2, P], BF16, kind="ExternalInput").ap()
    ident_d = nc.dram_tensor("ident", [P, P], BF16, kind="ExternalInput").ap()
    y_d = nc.dram_tensor("y", [TOK, C], BF16, kind="ExternalOutput").ap()
    dbg = {}
    if debug_taps:
        dbg["qT"] = nc.dram_tensor("dbg_qT", [P, TOK], F32, kind="ExternalOutput").ap()
        dbg["kT"] = nc.dram_tensor("dbg_kT", [P, TOK], F32, kind="ExternalOutput").ap()
        dbg["v"] = nc.dram_tensor("dbg_v", [P, B * KBB, 3, D], F32, kind="ExternalOutput").ap()
        dbg["attns"] = nc.dram_tensor("dbg_attns", [P, TOK], F32, kind="ExternalOutput").ap()

    with tile.TileContext(nc) as tc:
        with tc.tile_pool(name="res", bufs=1) as res, \
             tc.tile_pool(name="xt", bufs=2) as xtp, \
             tc.tile_pool(name="pt", bufs=4) as ptp, \
             tc.tile_pool(name="vt", bufs=4) as vtp, \
             tc.tile_pool(name="ys", bufs=8) as ysp:
            # ---- resident tensors ----
            wqk_sb = res.tile([P, 2, KO, P], BF16, tag="wqk")
            bqk_sb = res.tile([P, 2], F32, tag="bqk")
            wv_sb = res.tile([P, KO, P], BF16, tag="wv")
            wproj_sb = res.tile([P, C], BF16, tag="wproj")
            masks_sb = res.tile([P, 2, P], BF16, tag="masks")
            ident_sb = res.tile([P, P], BF16, tag="ident")

            qT_sb = res.tile([P, TOK], BF16, tag="qT")
            kT_sb = res.tile([P, TOK], BF16, tag="kT")
            # [v0 | ones | v1] per k-block; h0 reads cols 0:128, h1 64:192
            v_sb = res.tile([P, B * KBB, 3, D], BF16, tag="v")
            attns_sb = res.tile([P, TOK], BF16, tag="attns")

            # shared denominator-replicator ones block
            nc.vector.memset(v_sb[:, :, 1, :], 1.0)

            # ---- filler machinery: A(qkv) and C(proj) work is split into
            # small PE quanta pumped between attention j-steps, so the PE
            # (in-order queue) always has ready work while ScalarE runs exp.
            from collections import deque
            fill_a = deque()          # A quanta (qkv) — deadline-ordered
            fill_c = deque()          # C quanta (proj) — deadline-free
            a_left_box = [0]
            rr = [0]

            def pump(k=1):
                # alternate A/C so proj matmuls+DMAs spread through the
                # whole timeline instead of piling up at the tail
                n = 0
                while n < k and (fill_a or fill_c):
                    rr[0] += 1
                    src = fill_c if (fill_c and (rr[0] % 2 == 0 or not fill_a)) \
                        else fill_a
                    if src is fill_a:
                        a_left_box[0] -= 1
                    src.popleft()()
                    n += 1

            def pump_a(k=1):
                n = 0
                while n < k and fill_a:
                    a_left_box[0] -= 1
                    fill_a.popleft()()
                    n += 1

            # ---- A quanta: qkv for one 512-token chunk ----
            xt_tiles = {}   # pair -> list of 8 [P, 2*NQ] tiles

            def emit_xt_dma(pair, halves=False):
                # halves=True loads only the first 512 tokens of each tile
                # (chunk 0) so the first qk matmuls aren't starved by the
                # 728ns full-tile DMA cadence; the second halves follow via
                # emit_xt_dma2.
                xts = []
                for k in range(KO):
                    xt = xtp.tile([P, 2 * NQ], BF16, tag=f"xt{k}", name="xt")
                    sl = slice(0, NQ) if halves else slice(0, 2 * NQ)
                    nc.sync.dma_start(
                        xt[:, sl], xt_d[k * P:(k + 1) * P,
                                        pair * 2 * NQ + sl.start:
                                        pair * 2 * NQ + sl.stop])
                    xts.append(xt)
                xt_tiles[pair] = xts

            def emit_xt_dma2(pair):
                for k, xt in enumerate(xt_tiles[pair]):
                    nc.sync.dma_start(
                        xt[:, NQ:], xt_d[k * P:(k + 1) * P,
                                         pair * 2 * NQ + NQ:
                                         (pair + 1) * 2 * NQ])

            def make_A_quanta(n, psF):
                st = {}
                pair, half = n // 2, n % 2
                hs = slice(half * NQ, (half + 1) * NQ)

                def q_dma():
                    mark(f"A{n}.dma")
                    emit_xt_dma(pair)

                def q_qk(m):
                    def f():
                        mark(f"A{n}.qk{m}")
                        xts = xt_tiles[pair]
                        pq = psF.tile([P, NQ], F32, tag=f"f{m}", name="pq")
                        for k in range(KO):
                            nc.tensor.matmul(
                                pq[:], wqk_sb[:, m, k, :], xts[k][:, hs],
                                start=(k == 0), stop=(k == KO - 1))
                        dst = qT_sb if m == 0 else kT_sb
                        nc.vector.tensor_scalar_add(
                            dst[:, n * NQ:(n + 1) * NQ], pq[:],
                            bqk_sb[:, m:m + 1])
                    return f

                def q_vT():
                    # vT[feat, tok] accumulated with wv stationary, staged to
                    # SBUF for the PE transpose back to [token, feat]
                    mark(f"A{n}.vT")
                    xts = xt_tiles[pair]
                    pvT = psF.tile([P, NQ], F32, tag="f0", name="pvT")
                    for k in range(KO):
                        nc.tensor.matmul(
                            pvT[:], wv_sb[:, k, :], xts[k][:, hs],
                            start=(k == 0), stop=(k == KO - 1))
                    vt = vtp.tile([P, NQ], BF16, tag="vt", name="vt")
                    nc.vector.tensor_copy(vt[:], pvT[:])
                    st["vt"] = vt

                def q_tp(m2):
                    # PE-transpose one [128,128] block of vT back to natural
                    # [token, feat] layout; single copy scatters v0/v1 around
                    # the shared ones block
                    def f():
                        mark(f"A{n}.tp{m2}")
                        tp = psF.tile([P, P], BF16, tag=f"f{m2 % 2}", name="tp")
                        nc.tensor.transpose(
                            tp[:], st["vt"][:, m2 * P:(m2 + 1) * P], ident_sb[:])
                        kb = n * 4 + m2
                        nc.vector.tensor_copy(
                            v_sb[:, kb, 0:3:2, :],
                            tp[:].rearrange("p (h c) -> p h c", h=2))
                    return f

                qs = [q_dma] if (half == 0 and pair != 0) else []
                return qs + [q_qk(0), q_qk(1), q_vT,
                             q_tp(0), q_tp(1), q_tp(2), q_tp(3)]

            # ---- C quanta: output projection for one (m-block, half).
            # In tail mode (attention finished) the PSUM rotation widens
            # over the freed psS banks and copies alternate DVE/ScalarE.
            ys_tiles = {}
            c_state = {"n": 0, "alt": False, "tail": False}

            def make_C_quantum(m, n2):
                def f():
                    mark(f"C.m{m}.{n2}")
                    if n2 == 0:
                        ys_tiles[m] = ysp.tile([P, C], BF16, tag="ys", name="ys")
                    ys = ys_tiles[m]
                    cn = c_state["n"]
                    c_state["n"] += 1
                    if c_state["tail"]:
                        pool, tag = (psF, f"f{cn % 2}") \
                            if cn % 4 < 2 else (psS_g, "s")
                    else:
                        pool, tag = psF, f"f{cn % 2}"
                    # once ScalarE's exp backlog thins (last window onward),
                    # alternate copies over both engines so the in-order DVE
                    # queue can't stall the attention chain
                    eng = "act" if (c_state["alt"] and cn % 2) else "dve"
                    py = pool.tile([P, NQ], F32, tag=tag, name="py")
                    nc.tensor.matmul(
                        py[:], attns_sb[:, m * P:(m + 1) * P],
                        wproj_sb[:, n2 * NQ:(n2 + 1) * NQ],
                        start=True, stop=True)
                    if eng == "act":
                        nc.scalar.copy(ys[:, n2 * NQ:(n2 + 1) * NQ], py[:])
                    else:
                        nc.vector.tensor_copy(ys[:, n2 * NQ:(n2 + 1) * NQ], py[:])
                    if n2 == 1:
                        nc.sync.dma_start(y_d[m * P:(m + 1) * P, :], ys[:])
                        del ys_tiles[m]
                return f

            # ---- stage B: attention for batch b, token window [q0, q0+qw) ----
            js_left_box = [80]  # total j-steps over all B windows

            def emit_B(b, q0, qw, quiet=False):
                nq0 = b * T + q0
                jmax = (q0 + qw) // KB
                jdiag = q0 // KB   # j >= jdiag overlaps the q window
                psS, psO = psS_g, psO_g
                po = [psO.tile([P, NQ], F32, tag=f"o{h}", name=f"po{h}")
                      for h in range(2)]
                s_tiles = {}

                def emit_s(j):
                    # diagonal superblock: q-columns < j*KB - q0 are fully
                    # masked — skip them in S, exp and PV alike
                    c0 = max(0, j * KB - q0)
                    s = psS.tile([P, 2, NQ], F32, tag="s", name="s")
                    for h in range(2):
                        nc.tensor.matmul(
                            s[:, h, c0:qw],
                            kT_sb[h * D:(h + 1) * D,
                                  b * T + j * KB: b * T + (j + 1) * KB],
                            qT_sb[h * D:(h + 1) * D, nq0 + c0:nq0 + qw],
                            start=True, stop=True)
                    s_tiles[j] = (s, c0)

                emit_s(0)
                budget0 = (len(fill_a) + len(fill_c)) * jmax // js_left_box[0]
                js_left_box[0] -= jmax
                taken = 0
                for j in range(jmax):
                    mark(f"B{b}.{q0 // NQ}.j{j}")
                    if j + 1 < jmax:
                        emit_s(j + 1)
                    s, c0 = s_tiles.pop(j)
                    pt = ptp.tile([P, 2, NQ], BF16, tag="pt", name="pt")
                    if j >= jdiag:
                        # both heads' exp in one instruction (3D AP), then
                        # one merged triangle-mask multiply
                        nc.scalar.activation(pt[:, :, c0:qw], s[:, :, c0:qw], EXP)
                        nc.vector.tensor_mul(
                            pt[:, :, c0:c0 + KB], pt[:, :, c0:c0 + KB],
                            masks_sb[:])
                    else:
                        nc.scalar.activation(pt[:, :, 0:qw], s[:, :, 0:qw], EXP)
                    # cap filler per j-step so PV(j) never queues behind a
                    # long filler burst on the in-order PE queue; on the last
                    # step pump only after the norm ops are queued, so the
                    # chunk's normalization isn't stuck behind filler copies
                    if j + 1 < jmax and not quiet:
                        want = min(budget0 * (j + 1) // jmax, 3 * (j + 1))
                        if want > taken:
                            pump(want - taken)
                            taken = want
                    for h in range(2):
                        nc.tensor.matmul(
                            po[h][:, c0:qw], v_sb[:, b * KBB + j, h:h + 2, :],
                            pt[:, h, c0:qw],
                            start=(j == 0), stop=(j == jmax - 1))
                        if j == jmax - 1:
                            # normalize this head immediately: its recip
                            # runs on DVE while PE starts the other head.
                            # head1's [ones|v1] layout swaps num/den rows.
                            nd = D if h == 0 else 0
                            rc = ptp.tile([D, NQ], F32, tag="rc", name="rc")
                            nc.vector.reciprocal(rc[:, 0:qw], po[h][nd:nd + D, 0:qw])
                            nc.vector.tensor_mul(
                                attns_sb[h * D:(h + 1) * D, nq0:nq0 + qw],
                                po[h][D - nd:2 * D - nd, 0:qw], rc[:, 0:qw])
                if budget0 > taken and not quiet:
                    pump(budget0 - taken)

            # ---- interleaved emission ----
            with tc.tile_pool(name="psF", bufs=1, space="PSUM") as psF, \
                 tc.tile_pool(name="psS", bufs=2, space="PSUM") as psS_g, \
                 tc.tile_pool(name="psO", bufs=1, space="PSUM") as psO_g:
                # prologue DMAs, ordered so A0.qk0 can start earliest:
                # first the k=0..1 slice of wqk[m=0] and the k-ascending xt
                # tiles, so the first accumulation matmuls fire while the
                # rest of the weights stream in behind them.
                nc.sync.dma_start(wqk_sb[:, 0], wqk_d[:, 0])
                emit_xt_dma(0)
                nc.sync.dma_start(bqk_sb[:], bqk_d[:])
                nc.sync.dma_start(wqk_sb[:, 1], wqk_d[:, 1])
                nc.sync.dma_start(wv_sb[:], wv_d[:])
                nc.sync.dma_start(ident_sb[:], ident_d[:])
                nc.sync.dma_start(masks_sb[:], masks_d[:])
                nc.sync.dma_start(wproj_sb[:], wproj_d[:])

                a_total = 0
                a_prefix = [0]
                for n in range(NCHUNK):
                    qs = make_A_quanta(n, psF)
                    fill_a.extend(qs)
                    a_total += len(qs)
                    a_prefix.append(a_total)
                a_left_box[0] = a_total

                # batch-0 in order; batch-1 big windows first, then the
                # first 512 tokens as two 256-token sub-windows so the final
                # proj+DMA tail overlaps the last sub-window's attention
                windows = [(0, q0, NQ) for q0 in range(0, T, NQ)]
                windows += [(1, q0, NQ) for q0 in (0, NQ, 2 * NQ, 3 * NQ)]
                for wi, (b, q0, qw) in enumerate(windows):
                    last = wi == len(windows) - 1
                    if last:
                        c_state["alt"] = True
                    # A chunks needed by this window must be done first
                    need = a_total - a_prefix[(b * T + q0 + qw + NQ - 1) // NQ]
                    if a_left_box[0] > need:
                        pump_a(a_left_box[0] - need)
                    emit_B(b, q0, qw)
                    for mm in range(qw // P):
                        m = (b * T + q0) // P + mm
                        for n2 in range(2):
                            fill_c.append(make_C_quantum(m, n2))
                # trailing drain: attention is done, so the C rotation can
                # widen over the freed psS banks and use both copy engines
                c_state["tail"] = True
                pump_a(len(fill_a))
                while fill_c:
                    fill_c.popleft()()

            if debug_taps:
                dbg  # debug taps disabled in bf16 build

    nc.compile()
    return nc


def _host_prep(x, W_qkv, b_qkv, W_proj, b_proj):
    bf16 = ml_dtypes.bfloat16
    x = np.ascontiguousarray(np.asarray(x, dtype=np.float32))
    W_qkv = np.asarray(W_qkv, dtype=np.float32)
    b_qkv = np.asarray(b_qkv, dtype=np.float32)
    W_proj = np.asarray(W_proj, dtype=np.float32)
    b_proj = np.asarray(b_proj, dtype=np.float32)

    xT = np.ascontiguousarray(x.reshape(TOK, C).T.astype(bf16))  # [1024, 4096]
    scale = np.float32(1.0 / np.sqrt(D))

    masks = np.triu(np.ones((P, P), dtype=np.float32))  # [tk, tq]: tq >= tk
    masks2 = np.ascontiguousarray(
        np.stack([masks, masks], axis=1).astype(bf16))  # [P, 2, P]
    ident = np.ascontiguousarray(np.eye(P, dtype=np.float32).astype(bf16))

    in_maps = []
    for c in range(8):
        s0, s1 = c * P, (c + 1) * P
        wq = W_qkv[:, s0:s1] * scale
        wk = W_qkv[:, C + s0:C + s1]
        wv = W_qkv[:, 2 * C + s0:2 * C + s1]
        bq = b_qkv[s0:s1] * scale
        bk = b_qkv[C + s0:C + s1]
        # wqk host-rearranged to [p, m, ko, pcol] so DMAs are contiguous
        wqk = np.stack([wq, wk], axis=0)                 # [2, 1024, 128]
        wqk = wqk.reshape(2, KO, P, P).transpose(2, 0, 1, 3)  # [p, m, ko, pc]
        in_maps.append({
            "xt": xT,
            "wqk": np.ascontiguousarray(wqk.astype(bf16)),
            "bqk": np.ascontiguousarray(np.stack([bq, bk], axis=1)),
            "wv": np.ascontiguousarray(
                wv.reshape(KO, P, P).transpose(1, 0, 2).astype(bf16)),
            "wproj": np.ascontiguousarray(W_proj[s0:s1, :].astype(bf16)),
            "masks": masks2,
            "ident": ident,
        })
    # constant bias terms folded on host:
    #   out_proj bias + (v-bias row) @ W_proj  (v bias passes through softmax)
    ybias = b_qkv[2 * C:3 * C] @ W_proj + b_proj  # [1024]
    return in_maps, ybias


def kernel(x, W_qkv, b_qkv, W_proj, b_proj):
    if "nc" not in _CACHE:
        _CACHE["nc"] = _build()
    nc = _CACHE["nc"]
    in_maps, ybias = _host_prep(x, W_qkv, b_qkv, W_proj, b_proj)
    try:
        res = run_bass_kernel_spmd(nc, in_maps, core_ids=list(range(8)))
    except Exception:
        # transient device errors (NRT_EXEC_UNIT_UNRECOVERABLE) heal on retry
        res = run_bass_kernel_spmd(nc, in_maps, core_ids=list(range(8)))
    y = np.zeros((TOK, C), dtype=np.float32)
    for c in range(8):
        y += np.asarray(res.results[c]["y"]).astype(np.float32)
    y += ybias[None, :].astype(np.float32)
    return y.reshape(B, T, C)
